# revision 31
# baseline (speedup 1.0000x reference)
"""Multi-head attention kernel for 8 Trainium2 NeuronCores.

Problem: x[4, 2048, 1024], 16 heads x 64 head-dim MHA (QKV proj -> softmax
attention -> out proj), fp32 reference, rel-err gate 2e-2.

Sharding: 8 cores = 4 batches x 2 head-groups. Core c handles batch c//2 and
heads (c%2)*8 .. (c%2)*8+7. Each core computes a partial output [2048, 1024]
(its 8 heads through Wo); the host sums the two partials per batch and adds
bo + bv @ Wo (bias folds: bk drops out of softmax entirely; bv commutes
through the attention average into a constant output offset).

Precision design (validated in numpy sim, rel ~1.4e-2):
  - QKV projections: bf16 inputs (x^T and W pre-cast on host), fp32 psum.
  - Scores: fp8 DoubleRow matmul. Q is stored scaled by 8 with bias folded
    in as an fp8 (value, residual) pair; K is stored fp8 duplicated across
    the two DR k-slabs, so one DR matmul computes K8.(Q8a+Q8b) = K8.Qexact.
    Only the K-side fp8 error remains.
  - exp: stream A (even head-pairs) exact Exp on ScalarE -> fp8; stream B
    Schraudolph-style bit-trick on DVE: i8 = s*c1 + 56.5 cast to int8 IS the
    e4m3 bit pattern of ~exp(s/64). The constant-offset error cancels in the
    softmax normalization.
  - attn.V: two fp8 DoubleRow matmuls per key-pair against (V8, V-residual)
    tiles, killing the V quantization error. A 1/32 ones-column in the V8
    tile accumulates the softmax denominator in psum row 64.
  - ctx and out-projection: bf16 (fp8 ctx would cost 1.8e-2 alone).

Per-core engine usage: PE ~426k cycles of matmuls; exp work is split between
ScalarE (exact) and DVE (bit-trick); DVE also does residuals + normalize.
"""

import numpy as np
import ml_dtypes

B = 4
SEQ = 2048
DIM = 1024
NH_LOC = 8      # heads per core
HID = 64
HDL = NH_LOC * HID  # 512
N_CORES = 8

BF16 = ml_dtypes.bfloat16
E4M3 = ml_dtypes.float8_e4m3

# exp(s/64) ~= bits(s * SCHRAUD_MUL + SCHRAUD_OFF) as e4m3
SCHRAUD_MUL = float(8.0 * np.log2(np.e) / 64.0)
SCHRAUD_OFF = 56.0
EXP_SCALE = 1.0 / 64.0

_PROG = None


def _build_program(seq=SEQ, reps=1):
    import contextlib

    import concourse.bass as bass
    import concourse.mybir as mybir
    import concourse.tile as tile
    from concourse import bacc

    FP32 = mybir.dt.float32
    BF = mybir.dt.bfloat16
    F8 = mybir.dt.float8e4
    I8 = mybir.dt.int8
    Exp = mybir.ActivationFunctionType.Exp
    Ident = mybir.ActivationFunctionType.Identity
    Copy = mybir.ActivationFunctionType.Copy
    Alu = mybir.AluOpType
    DR = mybir.MatmulPerfMode.DoubleRow

    seq_t = seq // 128            # 16 key tiles
    n_qc = seq // 512             # 4 q chunks
    n_ktp = seq_t // 2            # 8 key-tile pairs
    dim_t = DIM // 128            # 8

    nc = bacc.Bacc()
    xt_d = nc.declare_dram_parameter("xt", [128, dim_t * seq], BF, isOutput=False)
    wq_d = nc.declare_dram_parameter("wq", [128, dim_t * HDL], BF, isOutput=False)
    wk_d = nc.declare_dram_parameter("wk", [128, dim_t * HDL], BF, isOutput=False)
    wv_d = nc.declare_dram_parameter("wv", [128, dim_t * HDL], BF, isOutput=False)
    wo_d = nc.declare_dram_parameter("wo", [128, 4 * DIM], BF, isOutput=False)
    bq_d = nc.declare_dram_parameter("bq8", [128, 4], FP32, isOutput=False)
    out_d = nc.declare_dram_parameter("out", [seq, DIM], BF, isOutput=True)
    rrs_d = nc.dram_tensor("rrs", [4, n_qc, 2, 512], FP32)

    env = dict(locals())

    with tile.TileContext(nc, pool_alloc_mode="queue") as tc:
        with tc.tile_pool(name="persist", bufs=1) as persist:
            xt_sb = persist.tile([128, dim_t * seq], BF, name="xt_sb")
            wq_sb = persist.tile([128, dim_t * HDL], BF, name="wq_sb")
            wk_sb = persist.tile([128, dim_t * HDL], BF, name="wk_sb")
            wv_sb = persist.tile([128, dim_t * HDL], BF, name="wv_sb")
            wo_sb = persist.tile([128, 4 * DIM], BF, name="wo_sb")
            bq_sb = persist.tile([128, 4], FP32, name="bq_sb")
            # Q fp8 (value, residual) pairs and K fp8 (dup) per head-pair m
            QP = [persist.tile([128, 2 * seq], F8, name=f"qp{m}") for m in range(4)]
            KD = [persist.tile([128, 2 * seq], F8, name=f"kd{m}") for m in range(4)]
            # V fp8 value/residual, keys-pair-slabbed, 65 cols per head
            VA = [persist.tile([128, 2 * NH_LOC * 80], F8, name=f"va{k}")
                  for k in range(n_ktp)]
            VB = [persist.tile([128, 2 * NH_LOC * 80], F8, name=f"vb{k}")
                  for k in range(n_ktp)]
            # 32*ctx in bf16: [128, o4, 512] per q-chunk
            UT = [persist.tile([128, 4 * 512], BF, name=f"ut{q}")
                  for q in range(n_qc)]

            rep_ctx = tc.For_i(0, reps, 1) if reps > 1 else contextlib.nullcontext()
            with rep_ctx:
                env.update(locals())
                _build_body(nc, tc, bass, mybir, tile, env)

    nc.compile()
    return nc


def _build_body(nc, tc, bass, mybir, tile, env):
    FP32 = mybir.dt.float32
    BF = mybir.dt.bfloat16
    F8 = mybir.dt.float8e4
    I8 = mybir.dt.int8
    Exp = mybir.ActivationFunctionType.Exp
    Ident = mybir.ActivationFunctionType.Identity
    Copy = mybir.ActivationFunctionType.Copy
    Alu = mybir.AluOpType
    DR = mybir.MatmulPerfMode.DoubleRow

    seq, seq_t, n_qc, n_ktp, dim_t = (env[k] for k in
        ["seq", "seq_t", "n_qc", "n_ktp", "dim_t"])
    xt_d, wq_d, wk_d, wv_d, wo_d, bq_d, out_d, rrs_d = (env[k] for k in
        ["xt_d", "wq_d", "wk_d", "wv_d", "wo_d", "bq_d", "out_d", "rrs_d"])
    xt_sb, wq_sb, wk_sb, wv_sb, wo_sb, bq_sb = (env[k] for k in
        ["xt_sb", "wq_sb", "wk_sb", "wv_sb", "wo_sb", "bq_sb"])
    QP, KD, VA, VB, UT = (env[k] for k in ["QP", "KD", "VA", "VB", "UT"])

    # ---------------- Phase A/B: load + QKV projections ----------------
    xtr = xt_sb[:].rearrange("p (d c) -> p d c", d=dim_t)
    xtdr = xt_d[:].rearrange("p (d c) -> p d c", d=dim_t)
    nc.sync.dma_start(out=bq_sb[:], in_=bq_d[:, :])
    nc.sync.dma_start(out=wv_sb[:], in_=wv_d[:, :])
    for sc in range(seq // 512):
        c0 = sc * 512
        (nc.scalar if sc % 2 == 0 else nc.sync).dma_start(
            out=xtr[:, :, c0:c0 + 512], in_=xtdr[:, :, c0:c0 + 512])
    nc.sync.dma_start(out=wk_sb[:], in_=wk_d[:, :])
    nc.sync.dma_start(out=wq_sb[:], in_=wq_d[:, :])
    nc.sync.dma_start(out=wo_sb[:], in_=wo_d[:, :])
    wqr = wq_sb[:].rearrange("p (d c) -> p d c", d=dim_t)
    wkr = wk_sb[:].rearrange("p (d c) -> p d c", d=dim_t)
    wvr = wv_sb[:].rearrange("p (d c) -> p d c", d=dim_t)
    qpr = [QP[m][:].rearrange("p (s c) -> p s c", s=2) for m in range(4)]
    kdr = [KD[m][:].rearrange("p (s c) -> p s c", s=2) for m in range(4)]
    var = [VA[k][:].rearrange("p (s h c) -> p s h c", s=2, h=NH_LOC)
           for k in range(n_ktp)]
    vbr = [VB[k][:].rearrange("p (s h c) -> p s h c", s=2, h=NH_LOC)
           for k in range(n_ktp)]
    utr = [UT[q][:].rearrange("p (o c) -> p o c", o=4) for q in range(n_qc)]

    with tc.tile_pool(name="qkvps", bufs=4, space="PSUM") as qkvps:
        # ones columns of the V tiles (row 64 of U = softmax denominator / 32)
        for k in range(n_ktp):
            nc.vector.memset(var[k][:, :, :, 64:65], 0.03125)
            nc.vector.memset(vbr[k][:, :, :, 64:65], 0.0)

        # V natural layout [seq, hd] + residual, strided into VA/VB
        for st in range(seq_t):
            vp = qkvps.tile([128, HDL], FP32, tag="qkv")
            for d in range(dim_t):
                nc.tensor.matmul(
                    vp[:], xtr[:, d, st * 128:(st + 1) * 128], wvr[:, d, :],
                    start=(d == 0), stop=(d == dim_t - 1))
            k, s = st // 2, st % 2
            vp3 = vp[:].rearrange("p (h c) -> p h c", c=HID)
            nc.scalar.activation(var[k][:, s, :, 0:HID], vp3, Copy)
            nc.vector.tensor_tensor(
                vbr[k][:, s, :, 0:HID], vp3, var[k][:, s, :, 0:HID],
                Alu.subtract)
        # K^T, duplicated across the two DR slabs via a stride-0 read
        for sc in range(seq // 512):
            c0 = sc * 512
            for m in range(4):
                kp = qkvps.tile([128, 512], FP32, tag="qkv")
                for d in range(dim_t):
                    nc.tensor.matmul(
                        kp[:], wkr[:, d, m * 128:(m + 1) * 128],
                        xtr[:, d, c0:c0 + 512],
                        start=(d == 0), stop=(d == dim_t - 1))
                kap = kp[:]
                kdup = bass.AP(tensor=kap.tensor, offset=kap.offset,
                               ap=[kap.ap[0], [0, 2], [1, 512]])
                nc.scalar.activation(kdr[m][:, :, c0:c0 + 512], kdup, Copy)
        # Q^T as fp8 (value, residual) pair, scaled by 8 with bias folded
        for sc in range(seq // 512):
            c0 = sc * 512
            for m in range(4):
                qp = qkvps.tile([128, 512], FP32, tag="qkv")
                for d in range(dim_t):
                    nc.tensor.matmul(
                        qp[:], wqr[:, d, m * 128:(m + 1) * 128],
                        xtr[:, d, c0:c0 + 512],
                        start=(d == 0), stop=(d == dim_t - 1))
                nc.scalar.activation(
                    qpr[m][:, 0, c0:c0 + 512], qp[:], Ident,
                    bias=bq_sb[:, m:m + 1], scale=1.0)
                nc.vector.scalar_tensor_tensor(
                    qpr[m][:, 1, c0:c0 + 512], qp[:], bq_sb[:, m:m + 1],
                    qpr[m][:, 0, c0:c0 + 512], Alu.add, Alu.subtract)

    # ---------------- Phase C: attention (single stream) ----------------
    # exp engine per kt: ACT-heavy split tuned so both engines load evenly
    # (DVE also runs recips, the normalize copies and Q/V residuals).
    EXP_ON_ACT = (True, False, True, True, False, True, True, False,
                  True, False, True, True, False, True, True, False)
    wor = wo_sb[:].rearrange("p (o c) -> p o c", o=4)
    with (
        tc.tile_pool(name="e2pool", bufs=4) as e2pool,
        tc.tile_pool(name="rpool", bufs=4) as rpool,
        tc.tile_pool(name="rbpool", bufs=4) as rbpool,
        tc.tile_pool(name="usbpool", bufs=3) as usbpool,
        tc.tile_pool(name="outstage", bufs=3) as outstage,
        tc.tile_pool(name="sps", bufs=3, space="PSUM") as sps,
        tc.tile_pool(name="ups", bufs=1, space="PSUM") as ups,
    ):
        def_mults = []    # (closure) broadcast+normalize from previous unit
        def_outs = []     # (closure) out-projection seq-tiles, interleaved



        def unit(m, qc):
            q0 = qc * 512
            us = [None, None]
            e2s = []

            def scores_exp(kt):
                s2 = sps.tile([128, 1024], FP32, tag="s2", name="s2")
                for h in range(2):
                    hb = h * 64
                    nc.tensor.matmul(
                        s2[:, h * 512:(h + 1) * 512],
                        kdr[m][hb:hb + 64, :, kt * 128:(kt + 1) * 128],
                        qpr[m][hb:hb + 64, :, q0:q0 + 512],
                        start=True, stop=True, perf_mode=DR)
                if kt % 2 == 0:
                    e2s.append(e2pool.tile([128, 2 * 1024], F8, tag="e2",
                                           name="e2"))
                e2 = e2s[kt // 2]
                b0 = (kt % 2) * 1024
                sl = e2[:, b0:b0 + 1024]
                if EXP_ON_ACT[kt]:
                    nc.scalar.activation(sl, s2[:], Exp, scale=EXP_SCALE)
                else:
                    nc.vector.tensor_scalar(
                        sl.bitcast(mybir.dt.int8), s2[:],
                        SCHRAUD_MUL, SCHRAUD_OFF, Alu.mult, Alu.add)

            def attn_half(ktp, h):
                # one head's (V8a + V8b) DR accumulation for key-pair ktp
                if us[h] is None:
                    us[h] = ups.tile([65, 512], FP32, tag=f"u{h}",
                                     name=f"u{h}")
                e2r = e2s[ktp][:].rearrange("p (s c) -> p s c", s=2)
                hg = 2 * m + h
                rhs = e2r[:, :, h * 512:(h + 1) * 512]
                nc.tensor.matmul(
                    us[h][:], var[ktp][:, :, hg, 0:65], rhs,
                    start=(ktp == 0), stop=False, perf_mode=DR)
                nc.tensor.matmul(
                    us[h][:], vbr[ktp][:, :, hg, 0:65], rhs,
                    start=False, stop=(ktp == n_ktp - 1), perf_mode=DR)

            def finish():
                # Immediately: reciprocals (from psum row 64) and U body
                # copies to SBUF, releasing the psum banks fast. The
                # broadcast DMA + single combined normalize mult are
                # deferred into the next unit (rb latency hidden; nothing
                # left that gates psum reuse).
                for h in range(2):
                    rr = rpool.tile([1, 512], FP32, tag="rr")
                    nc.vector.reciprocal(rr[:], us[h][64:65, :])
                    nc.sync.dma_start(out=rrs_d[m, qc, h, :], in_=rr[0:1, :])
                usb = usbpool.tile([128, 512], FP32, tag="usb")
                nc.scalar.activation(usb[0:64, :], us[0][0:HID, :], Copy)
                nc.vector.tensor_copy(usb[64:128, :], us[1][0:HID, :])

                def mult():
                    rb = rbpool.tile([128, 512], FP32, tag="rb")
                    slot2 = rrs_d[m, qc, :, :]
                    nc.sync.dma_start(
                        out=rb[:],
                        in_=bass.AP(tensor=slot2.tensor, offset=slot2.offset,
                                    ap=[[512, 2], [0, 64], [1, 512]]))
                    nc.vector.tensor_tensor(
                        utr[qc][:, m, :], usb[:], rb[:], Alu.mult)
                def_mults.append(mult)
            return scores_exp, attn_half, finish

        for qc in range(n_qc):
            for m in range(4):
                se, ah, fi = unit(m, qc)
                for kt in range(seq_t):
                    se(kt)
                    # previous unit's deferred broadcast + normalize
                    if kt == 2 and def_mults:
                        def_mults.pop(0)()
                    if kt >= 3:
                        idx = kt - 3
                        ah(idx // 2, idx % 2)
                for idx in range(seq_t - 3, seq_t):
                    ah(idx // 2, idx % 2)
                fi()
        while def_mults:
            def_mults.pop(0)()

    # ---------------- Phase D: out projection (bf16) ----------------
    with (
        tc.tile_pool(name="outstage", bufs=3) as outstage2,
        tc.tile_pool(name="ops", bufs=4, space="PSUM") as ops,
    ):
        for st in range(seq_t):
            qc, c0 = st // 4, (st % 4) * 128
            ot = outstage2.tile([128, DIM], BF, tag="ot")
            for oc in range(2):
                op_t = ops.tile([128, 512], FP32, tag="op")
                for o in range(4):
                    nc.tensor.matmul(
                        op_t[:], utr[qc][:, o, c0:c0 + 128],
                        wor[:, o, oc * 512:(oc + 1) * 512],
                        start=(o == 0), stop=(o == 3))
                # UT holds 32*ctx -> scale back here
                if oc == 0:
                    nc.scalar.activation(ot[:, oc * 512:(oc + 1) * 512],
                                         op_t[:], Copy, scale=0.03125)
                else:
                    nc.vector.tensor_scalar(ot[:, oc * 512:(oc + 1) * 512],
                                            op_t[:], 0.03125, None, Alu.mult)
            nc.sync.dma_start(out=out_d[st * 128:(st + 1) * 128, :], in_=ot[:])


def _get_program():
    global _PROG
    if _PROG is None:
        _PROG = _build_program()
    return _PROG


def _prep_core_inputs(x, Wq, bq, Wk, Wv, Wo):
    """Host-side layout/cast for one core: x [2048, 1024] f32, W* pre-sliced."""
    xt = np.ascontiguousarray(x.T).astype(BF16)            # [1024, 2048]
    xt = xt.reshape(8, 128, SEQ).transpose(1, 0, 2).reshape(128, 8 * SEQ)
    def wlay(w):                                           # [1024, 512]
        w = np.asarray(w, np.float32).astype(BF16)
        return w.reshape(8, 128, HDL).transpose(1, 0, 2).reshape(128, 8 * HDL)
    wo = np.asarray(Wo, np.float32).astype(BF16)           # [512, 1024]
    wo = wo.reshape(4, 128, DIM).transpose(1, 0, 2).reshape(128, 4 * DIM)
    bq8 = np.ascontiguousarray(
        (8.0 * np.asarray(bq, np.float32)).reshape(4, 128).T)
    return {
        "xt": np.ascontiguousarray(xt),
        "wq": np.ascontiguousarray(wlay(8.0 * np.asarray(Wq, np.float32))),
        "wk": np.ascontiguousarray(wlay(Wk)),
        "wv": np.ascontiguousarray(wlay(Wv)),
        "wo": np.ascontiguousarray(wo),
        "bq8": bq8,
    }


def _make_in_maps(inputs):
    x = np.asarray(inputs["x"], dtype=np.float32)
    Wq = np.asarray(inputs["Wq"], np.float32)
    Wk = np.asarray(inputs["Wk"], np.float32)
    Wv = np.asarray(inputs["Wv"], np.float32)
    Wo = np.asarray(inputs["Wo"], np.float32)
    bq = np.asarray(inputs["bq"], np.float32)
    in_maps = []
    for c in range(N_CORES):
        b, g = divmod(c, 2)
        sl = slice(g * HDL, (g + 1) * HDL)
        in_maps.append(_prep_core_inputs(
            x[b], Wq[:, sl], bq[sl], Wk[:, sl], Wv[:, sl], Wo[sl, :]))
    return in_maps


def kernel(x, Wq, bq, Wk, bk, Wv, bv, Wo, bo):
    from concourse.bass_utils import run_bass_kernel_spmd

    bo = np.asarray(bo, dtype=np.float32)
    bv = np.asarray(bv, dtype=np.float32)
    Wo_f = np.asarray(Wo, np.float32)
    nc = _get_program()
    in_maps = _make_in_maps(dict(x=x, Wq=Wq, bq=bq, Wk=Wk, Wv=Wv, Wo=Wo))
    res = run_bass_kernel_spmd(nc, in_maps, core_ids=list(range(N_CORES)))
    extra = bo + bv @ Wo_f           # bv folds through the attention average
    out = np.empty((B, SEQ, DIM), dtype=np.float32)
    for b in range(B):
        out[b] = (res.results[2 * b]["out"].astype(np.float32)
                  + res.results[2 * b + 1]["out"].astype(np.float32) + extra)
    return out
